# revision 11
# baseline (speedup 1.0000x reference)
"""Multi-head self-attention TRN2 Bass kernel (v2).

Problem: B=4, S=2048, EMB=1024, H=16 heads, dqk=dv=64.
Sharding: 8 cores; core c handles batch b=c//2 and head group g=c%2
(8 heads each). Each core computes its partial output projection
(rows of wo for its heads); host sums the two partials per batch and
adds bo + (bv @ wo) (the V-bias term commutes through attention's
convex average, so it is applied on the host exactly).

v2 changes vs v1:
  - Projections are interleaved into the attention t-loops instead of
    running as a 100us PE-only prologue (ScalarE idled there).
  - AV matmuls lag scores by LAG t-steps so each block's softmax-
    normalize tail overlaps the next block's score matmuls (v1 stalled
    the PE ~10us per block, re-throttling HAM to 1.2GHz).
  - The per-(par,i) [1,512] DVE reciprocals (3.3us each: 1 lane x
    8cyc/elem) are replaced by one [128,16] reciprocal on DMA-reshaped
    denominators.
  - Scores matmul pairs are emitted par-interleaved so the two K=64
    head matmuls (row groups 0-63 / 64-127) overlap in the PE array.
  - bv is folded into the host-side constant (bv @ wo), removing the
    device bias adds and shortening the tail chain.

Dataflow per core:
  - Q/K projections j-major: QHT/KHT [512 j, 2048 s] bf16.
  - V projection t-major: VH [2048 t, 512 j] as tiles [128, 8*66] with
    per-head blocks [64 V | 1 ones | 1 pad]; the ones column makes the
    AV matmul emit softmax denominators as row 64 of its output.
  - scores (t-major, K=64 row-tiled): S^T [128 t, 1024 s] PSUM, exp on
    ACT (scale=1/8 folded; max-free softmax, |scores| <~ 3) -> P^T bf16.
  - AV: accumulate Z~T [66, 512] over 16 t-tiles in PSUM.
  - normalize: denominators DMA-reshaped to [128,16], one reciprocal,
    DMA back + broadcast via DRAM; multiply -> ZnormT bf16 (par0 heads
    write znorm[0:64] directly, par1 staged + SBUF DMA to [64:128]).
  - out projection bf16 -> partial OUT [2048, 1024] f32 -> DRAM.
"""

import ml_dtypes
import numpy as np

import concourse.bass as bass
import concourse.tile as tile
from concourse import bacc, mybir
from concourse.bass_utils import run_bass_kernel_spmd

B, S, EMB, H, DH = 4, 2048, 1024, 16, 64
N_CORES = 8
HPC = H // 2          # heads per core
JC = HPC * DH         # 512: per-core projected width
VB = DH + 2           # 66: per-head V block (64 V cols + ones + pad)
LAG = 4               # AV lags scores by this many t-steps

F32 = mybir.dt.float32
BF16 = mybir.dt.bfloat16


def build_kernel(reps=1):
    nc = bacc.Bacc(
        "TRN2", target_bir_lowering=False, debug=False, num_devices=N_CORES
    )

    xq = nc.dram_tensor("xq", [EMB, S], BF16, kind="ExternalInput").ap()
    xkv = nc.dram_tensor("xkv", [EMB, S], BF16, kind="ExternalInput").ap()
    wq_d = nc.dram_tensor("wq", [EMB, JC], BF16, kind="ExternalInput").ap()
    wk_d = nc.dram_tensor("wk", [EMB, JC], BF16, kind="ExternalInput").ap()
    wv_d = nc.dram_tensor("wv", [EMB, JC], BF16, kind="ExternalInput").ap()
    bq_d = nc.dram_tensor("bq", [JC], F32, kind="ExternalInput").ap()
    bk_d = nc.dram_tensor("bk", [JC], F32, kind="ExternalInput").ap()
    wo_d = nc.dram_tensor("wo", [JC, EMB], BF16, kind="ExternalInput").ap()
    out_d = nc.dram_tensor("out", [S, EMB], F32, kind="ExternalOutput").ap()
    dr_d = nc.dram_tensor("dr_scratch", [2048], F32).ap()   # D rows, flat
    dr2_d = nc.dram_tensor("dr2_scratch", [2, 1024], F32).ap()  # 1/D per par

    import contextlib

    with tile.TileContext(nc) as tc:
        with (
            tc.For_i(0, reps, 1) if reps > 1 else contextlib.nullcontext(),
            tc.tile_pool(name="persist", bufs=1) as pp,
        ):
            # persistent SBUF tensors
            qht = [pp.tile([128, S], BF16, name=f"qht{i}") for i in range(4)]
            kht = [pp.tile([128, S], BF16, name=f"kht{i}") for i in range(4)]
            vh = [pp.tile([128, HPC * VB], BF16, name=f"vh{t}")
                  for t in range(16)]
            znorm = [pp.tile([128, S], BF16, name=f"zn{i}") for i in range(4)]
            wos = [pp.tile([128, EMB], BF16, name=f"wo{j}") for j in range(4)]
            bias_q = pp.tile([128, 4], F32, name="bias_q")
            bias_k = pp.tile([128, 4], F32, name="bias_k")
            wqs = [pp.tile([128, JC], BF16, name=f"wqs{e}") for e in range(8)]
            wks = [pp.tile([128, JC], BF16, name=f"wks{e}") for e in range(8)]
            wvs = [pp.tile([128, JC], BF16, name=f"wvs{e}") for e in range(8)]
            xqr = [pp.tile([128, S], BF16, name=f"xqr{e}") for e in range(8)]
            xkr = [pp.tile([128, S], BF16, name=f"xkr{e}") for e in range(8)]

            # DMA in first-use order
            nc.sync.dma_start(bias_q[:], bq_d.rearrange("(c p) -> p c", p=128))
            nc.sync.dma_start(bias_k[:], bk_d.rearrange("(c p) -> p c", p=128))
            for e in range(8):
                nc.sync.dma_start(wqs[e][:], wq_d[e * 128:(e + 1) * 128, :])
            for e in range(8):
                nc.sync.dma_start(xqr[e][:, 0:1024],
                                  xq[e * 128:(e + 1) * 128, 0:1024])
            for e in range(8):
                nc.sync.dma_start(wks[e][:], wk_d[e * 128:(e + 1) * 128, :])
            for e in range(8):
                nc.sync.dma_start(xkr[e][:, 0:1024],
                                  xkv[e * 128:(e + 1) * 128, 0:1024])
            for e in range(8):
                nc.sync.dma_start(xqr[e][:, 1024:2048],
                                  xq[e * 128:(e + 1) * 128, 1024:2048])
            for e in range(8):
                nc.sync.dma_start(xkr[e][:, 1024:2048],
                                  xkv[e * 128:(e + 1) * 128, 1024:2048])
            for e in range(8):
                nc.sync.dma_start(wvs[e][:], wv_d[e * 128:(e + 1) * 128, :])
            for j in range(4):
                nc.sync.dma_start(wos[j][:], wo_d[j * 128:(j + 1) * 128, :])
            # ones columns in vh blocks (col 64 of each 66-block); pad col 0
            for t in range(16):
                blocks = vh[t][:].rearrange("p (h c) -> p h c", c=VB)
                nc.vector.memset(blocks[:, :, DH:DH + 1], 1.0)
                nc.vector.memset(blocks[:, :, DH + 1:], 0.0)

            with (
                tc.tile_pool(name="sps", bufs=1, space="PSUM") as sp_pool,
                tc.tile_pool(name="avps", bufs=1, space="PSUM") as av_pool,
                tc.tile_pool(name="pt", bufs=5) as pt_pool,
                tc.tile_pool(name="dre", bufs=1) as dre_pool,
                tc.tile_pool(name="znsc", bufs=2) as zns_pool,
            ):
                chunk_ctr = [0]

                def proj_chunk(kind, pair_or_t, sc=0):
                    """Emit one projection chunk (8 MMs + epilogue)."""
                    tagi = chunk_ctr[0] % 2
                    chunk_ctr[0] += 1
                    ps = sp_pool.tile([128, 1024], F32, tag=f"sp{tagi}",
                                      name=f"sp{tagi}")
                    if kind == "v":
                        tch = pair_or_t
                        tsl = slice(tch * 128, (tch + 1) * 128)
                        for e in range(8):
                            nc.tensor.matmul(
                                ps[:, 0:512], xkr[e][:, tsl], wvs[e][:],
                                start=(e == 0), stop=(e == 7),
                            )
                        nc.vector.tensor_copy(
                            vh[tch][:].rearrange(
                                "p (h c) -> p h c", c=VB)[:, :, 0:DH],
                            ps[:, 0:512].rearrange("p (h d) -> p h d", d=DH),
                        )
                    else:
                        pair = pair_or_t
                        ws, xr, dst, bias = (
                            (wqs, xqr, qht, bias_q) if kind == "q"
                            else (wks, xkr, kht, bias_k)
                        )
                        jsl = slice(pair * 128, (pair + 1) * 128)
                        ssl = slice(sc * 512, (sc + 1) * 512)
                        for e in range(8):
                            nc.tensor.matmul(
                                ps[:, 0:512], ws[e][:, jsl], xr[e][:, ssl],
                                start=(e == 0), stop=(e == 7),
                            )
                        nc.vector.tensor_scalar_add(
                            dst[pair][:, ssl], ps[:, 0:512],
                            bias[:, pair:pair + 1],
                        )

                def attn_block(pair, s_half, work):
                    """One (pair, s_half) attention block; `work` is a list
                    of proj-chunk thunks interleaved one per t-step."""
                    s0 = s_half * 1024
                    avs = {}
                    for par in range(2):
                        for i in range(2):
                            avs[(par, i)] = av_pool.tile(
                                [VB, 512], F32, tag=f"av{par}{i}",
                                name=f"av{par}{i}")

                    pts = {}

                    def emit_scores(t):
                        # sp tiles are per s-chunk i: cols [0:512]=par0,
                        # [512:1024]=par1.  One ACT covers both pars, so
                        # both next-t score MMs unblock together and the
                        # K=64 pair (row groups 0-63 / 64-127) runs
                        # concurrently in the PE array.
                        for i in range(2):
                            ps = sp_pool.tile([128, 1024], F32,
                                              tag=f"sp{i}", name=f"sp{i}")
                            for par in range(2):
                                off = par * 64
                                nc.tensor.matmul(
                                    ps[:, par * 512:(par + 1) * 512],
                                    kht[pair][off:off + 64,
                                              t * 128:(t + 1) * 128],
                                    qht[pair][off:off + 64,
                                              s0 + i * 512:s0 + (i + 1) * 512],
                                    start=True, stop=True,
                                )
                            ptt = pt_pool.tile([128, 1024], BF16,
                                               tag=f"pt{i}", name=f"pt{i}")
                            nc.scalar.activation(
                                ptt[:], ps[:],
                                mybir.ActivationFunctionType.Exp,
                                scale=0.125,
                            )
                            pts[(t, i)] = ptt

                    def emit_av(t):
                        for par in range(2):
                            h = pair * 2 + par
                            for i in range(2):
                                nc.tensor.matmul(
                                    avs[(par, i)][:],
                                    vh[t][:, h * VB:(h + 1) * VB],
                                    pts[(t, i)][:, par * 512:(par + 1) * 512],
                                    start=(t == 0), stop=(t == 15),
                                    skip_group_check=True,
                                )
                        pts.pop((t, 0))
                        pts.pop((t, 1))

                    for t in range(16):
                        emit_scores(t)
                        if t < len(work):
                            work[t]()
                        if t >= LAG:
                            emit_av(t - LAG)
                    for t in range(16 - LAG, 16):
                        emit_av(t)

    # ---- tail: reciprocal of D + normalize ----
                    # D rows (PSUM row 64 of each av bank) -> SBUF row,
                    # then DMA-reshape across partitions for the recip
                    dsb = dre_pool.tile([65, 2048], F32, tag="dsb",
                                        name="dsb")
                    for par in range(2):
                        for i in range(2):
                            k = par * 2 + i
                            nc.vector.tensor_copy(
                                dsb[DH:DH + 1, k * 512:(k + 1) * 512],
                                avs[(par, i)][DH:DH + 1, :],
                            )
                    nc.sync.dma_start(dr_d[:], dsb[DH:DH + 1, :])
                    recdt = dre_pool.tile([128, 16], F32, tag="rdt",
                                          name="recdt")
                    nc.sync.dma_start(
                        recdt[:], dr_d.rearrange("(p c) -> p c", p=128))
                    recdr = dre_pool.tile([128, 16], F32, tag="rdr",
                                          name="recdr")
                    nc.vector.reciprocal(recdr[:], recdt[:])
                    # scatter back: flat idx p*16+c == par*1024 + i*512 + s
                    nc.sync.dma_start(
                        dr2_d.rearrange("a (b c) -> (a b) c", c=16),
                        recdr[:],
                    )
                    for par in range(2):
                        dreb = dre_pool.tile([64, 1024], F32,
                                             tag=f"db{par}",
                                             name=f"db{par}")
                        nc.sync.dma_start(
                            dreb[:],
                            dr2_d[par:par + 1, :].broadcast_to([64, 1024]),
                        )
                        if par == 0:
                            for i in range(2):
                                nc.vector.tensor_mul(
                                    znorm[pair][0:DH,
                                                s0 + i * 512:
                                                s0 + (i + 1) * 512],
                                    avs[(par, i)][0:DH, :],
                                    dreb[:, i * 512:(i + 1) * 512],
                                )
                        else:
                            zn_s = zns_pool.tile([64, 1024], BF16,
                                                 tag="zns", name="zn_s")
                            for i in range(2):
                                nc.vector.tensor_mul(
                                    zn_s[:, i * 512:(i + 1) * 512],
                                    avs[(par, i)][0:DH, :],
                                    dreb[:, i * 512:(i + 1) * 512],
                                )
                            nc.sync.dma_start(
                                znorm[pair][DH:2 * DH, s0:s0 + 1024],
                                zn_s[:],
                            )

                with tc.tile_pool(name="ostg", bufs=2) as ostg_pool:

                    def outproj_chunk(scc):
                        """One 128-row slice of the output projection,
                        using half of an sp slot per oc."""
                        tagi = chunk_ctr[0] % 2
                        chunk_ctr[0] += 1
                        ps = sp_pool.tile([128, 1024], F32, tag=f"sp{tagi}",
                                          name=f"sp{tagi}")
                        psl = slice(scc * 128, (scc + 1) * 128)
                        for jt in range(4):
                            for oc in range(2):
                                nc.tensor.matmul(
                                    ps[:, oc * 512:(oc + 1) * 512],
                                    znorm[jt][:, psl],
                                    wos[jt][:, oc * 512:(oc + 1) * 512],
                                    start=(jt == 0),
                                    stop=(jt == 3),
                                    skip_group_check=True,
                                )
                        for oc in range(2):
                            osl = slice(oc * 512, (oc + 1) * 512)
                            ostg = ostg_pool.tile([128, 512], F32,
                                                  tag="ostg", name="ostg")
                            nc.vector.tensor_copy(
                                ostg[:], ps[:, oc * 512:(oc + 1) * 512])
                            nc.sync.dma_start(out_d[psl, osl], ostg[:])

                    # ---- front matter: Q/K proj for pair 0 ----
                    for sc in range(4):
                        proj_chunk("q", 0, sc)
                    for sc in range(4):
                        proj_chunk("k", 0, sc)

                    # proj work lists per block index (pair*2 + s_half);
                    # spread so every block keeps the PE busy (idle-ish
                    # blocks let HAM re-throttle the PE clock to 1.2GHz)
                    work_lists = [[] for _ in range(8)]
                    for tch in range(16):
                        work_lists[0].append(
                            lambda tch=tch: proj_chunk("v", tch))
                    for p, blks in ((1, (1, 1)), (2, (2, 3)), (3, (4, 5))):
                        for k, (kind, sc) in enumerate(
                                [("k", s) for s in range(4)]
                                + [("q", s) for s in range(4)]):
                            wl = blks[0] if k < 4 else blks[1]
                            # Q sc2/sc3 are only needed by (p, s_half=1):
                            # push pair-3's to block 6 for better balance
                            if p == 3 and kind == "q" and sc >= 2:
                                wl = 6
                            work_lists[wl].append(
                                lambda kind=kind, p=p, sc=sc:
                                proj_chunk(kind, p, sc))
                    # output projection for s_half 0 hides in block 7;
                    # delay a few t-steps so block 6's normalize tail
                    # (last znorm[3] sh0 writes) has landed
                    for _ in range(4):
                        work_lists[7].append(lambda: None)
                    for scc in range(8):
                        work_lists[7].append(
                            lambda scc=scc: outproj_chunk(scc))

                    for pair in range(4):
                        for s_half in range(2):
                            attn_block(pair, s_half,
                                       work_lists[pair * 2 + s_half])

                    # ---- output projection, s_half 1 ----
                    for scc in range(8, 16):
                        outproj_chunk(scc)

    nc.compile()
    return nc


def _bf16(a):
    return np.asarray(a, np.float32).astype(ml_dtypes.bfloat16)


def _prep_inputs(q, k_and_v, wq, bq, wk, bk, wv, bv, wo):
    """Build per-core input maps."""
    in_maps = []
    for c in range(N_CORES):
        b, g = c // 2, c % 2
        hs = slice(g * HPC, (g + 1) * HPC)
        # [H, emb, d] -> [emb, H*d] for this head group
        wq_g = np.transpose(wq[hs], (1, 0, 2)).reshape(EMB, JC)
        wk_g = np.transpose(wk[hs], (1, 0, 2)).reshape(EMB, JC)
        wv_g = np.transpose(wv[hs], (1, 0, 2)).reshape(EMB, JC)
        in_maps.append({
            "xq": np.ascontiguousarray(_bf16(q[b]).T),
            "xkv": np.ascontiguousarray(_bf16(k_and_v[b]).T),
            "wq": np.ascontiguousarray(_bf16(wq_g)),
            "wk": np.ascontiguousarray(_bf16(wk_g)),
            "wv": np.ascontiguousarray(_bf16(wv_g)),
            "bq": np.ascontiguousarray(np.asarray(bq, np.float32)[hs]
                                       .reshape(JC)),
            "bk": np.ascontiguousarray(np.asarray(bk, np.float32)[hs]
                                       .reshape(JC)),
            "wo": np.ascontiguousarray(
                _bf16(wo)[g * JC:(g + 1) * JC, :]),
        })
    return in_maps


def _host_bias(bv, wo, bo):
    """bo + bv @ wo (the V-bias commutes through the attention average)."""
    bv_f = np.asarray(bv, np.float64).reshape(H * DH)
    wo_f = np.asarray(wo, np.float64).reshape(H * DH, EMB)
    return (np.asarray(bo, np.float64) + bv_f @ wo_f).astype(np.float32)


_NC_CACHE = {}


def kernel(q, k_and_v, wq, bq, wk, bk, wv, bv, wo, bo):
    if "nc" not in _NC_CACHE:
        _NC_CACHE["nc"] = build_kernel()
    nc = _NC_CACHE["nc"]
    in_maps = _prep_inputs(q, k_and_v, wq, bq, wk, bk, wv, bv, wo)
    res = run_bass_kernel_spmd(nc, in_maps, core_ids=list(range(N_CORES)))
    hb = _host_bias(bv, wo, bo)
    out = np.empty((B, S, EMB), np.float32)
    for b in range(B):
        out[b] = res.results[2 * b]["out"] + res.results[2 * b + 1]["out"] + hb
    return out
